# revision 1
# baseline (speedup 1.0000x reference)
"""Trainium2 Bass kernel for nn_Attention_9242769622327.

Math: the reference computes
    qkv = x @ W1.T ; q,k,v = split(qkv)
    score = softmax(k^T v / 4, axis=-1)            # rows sum to 1
    attn  = softmax(einsum('bhnk,bhkc->bhnk', q/4, score), axis=-1)
          = softmax(q/4 * sum_c score)             # sum_c score == 1
          = softmax(q/4)                           # k/v are mathematically dead
    out   = attn @ W2.T
so only the q-projection (first E rows of W1), a per-head (64-wide) softmax,
and the output projection are needed.

Distribution: pure data-parallel over the 32768 = B*S rows; each of the 8
cores handles 4096 rows with the full (transposed, fp16) weights. No
collectives.  fp16 runs the PE at the same 1 cycle/row as bf16 but with a
10-bit mantissa (rel err ~4.5e-4 vs ~3.6e-3 for bf16).

On-chip layout is fully transposed (features on partitions, rows on the free
dim) so no on-chip transposes are needed anywhere:
    qT[n,m]  = sum_k W1qT[k,n] * xT[k,m]          (PE, fp16)
    u        = exp(qT/4)                          (ACT, PSUM->SBUF fp16)
    s[g,m]   = sum_{n in head g} u[n,m]           (PE matmul w/ 0/1 selector)
    rcp      = 1/s                                (DVE reciprocal_approx_fast;
                                                   no Ln -> one ACT table set)
    rb[n,m]  = rcp[head(n),m]                     (PE matmul w/ selector^T,
                                                   K padded to 128 so LDW overlaps)
    aT       = u * rb                             (DVE)
    outT[j,m]= sum_n W2T[n,j] * aT[n,m]           (PE, fp16)

Stripes are software-pipelined: stripe ms runs [64 mm1][8 rb(ms-1)][8 sel]
[64 mm2(ms-1)] as contiguous same-shape matmul blocks on the PE (back-to-back
216ns issue at N=512), with exp/reciprocal/normalize hidden underneath.
Measured: 280.8us on 8 cores, rel err 4.5e-4 (vs ~249us pure-matmul floor).
"""

import sys

sys.path.insert(0, "/opt/trn_rl_repo")

import numpy as np
import ml_dtypes

import concourse.bass as bass
import concourse.bacc as bacc
import concourse.tile as tile
from concourse import mybir
from concourse.bass_utils import run_bass_kernel_spmd

BF16 = mybir.dt.float16  # fp16: same PE rate as bf16, 10-bit mantissa
F32 = mybir.dt.float32
AF = mybir.ActivationFunctionType

N_CORES = 8
B, S, E = 4, 8192, 1024
HEADS, HEAD_DIM = 16, 64
M_TOTAL = B * S                # 32768
M_CORE = M_TOTAL // N_CORES    # 4096 rows per core
MS = 512                       # m-stripe width (moving free dim / PSUM bank)
N_STRIPES = M_CORE // MS       # 8
KC = E // 128                  # 8 contraction chunks
NC_ = E // 128                 # 8 feature chunks

_BF = np.float16


def build_nc() -> bass.Bass:
    nc = bacc.Bacc("TRN2", debug=False)

    xt = nc.dram_tensor("xt", [E, M_CORE], BF16, kind="ExternalInput")
    w1t = nc.dram_tensor("w1t", [E, E], BF16, kind="ExternalInput")
    w2t = nc.dram_tensor("w2t", [E, E], BF16, kind="ExternalInput")
    sel = nc.dram_tensor("sel", [128, NC_ * HEADS], BF16, kind="ExternalInput")
    selt = nc.dram_tensor("selt", [128, NC_ * 128], BF16, kind="ExternalInput")
    outT = nc.dram_tensor("outT", [E, M_CORE], BF16, kind="ExternalOutput")

    xt_v = xt[:, :].rearrange("(c p) m -> p c m", p=128)    # [128, 8, M_CORE]
    w1_v = w1t[:, :].rearrange("(c p) n -> p c n", p=128)   # [128, 8, 1024]
    w2_v = w2t[:, :].rearrange("(c p) j -> p c j", p=128)   # [128, 8, 1024]

    with tile.TileContext(nc) as tc:
        with (
            tc.tile_pool(name="weights", bufs=1) as wpool,
            tc.tile_pool(name="xt", bufs=N_STRIPES) as xpool,
            tc.tile_pool(name="u", bufs=16) as upool,
            tc.tile_pool(name="at", bufs=16) as apool,
            tc.tile_pool(name="small", bufs=3) as spool,
            tc.tile_pool(name="ostage", bufs=8) as opool,
            tc.tile_pool(name="ps_q", bufs=2, space="PSUM") as psq,
            tc.tile_pool(name="ps_s", bufs=2, space="PSUM") as pss,
            tc.tile_pool(name="ps_rb", bufs=2, space="PSUM") as psrb,
            tc.tile_pool(name="ps_o", bufs=2, space="PSUM") as pso,
        ):
            # Per-chunk weight tiles so the first matmuls only wait on the
            # chunks they read, not the whole 4MB of weights.  Load order:
            # w1 + sel (needed by stripe 0's mm1/sel), stripe-0 x chunks,
            # then w2 + selt (not needed until ~18us in).
            # Warm the PE's HAM clock gate with throwaway matmuls on memset
            # scratch while the first weight/x DMAs are in flight, so the
            # first real matmuls run at 2.4 GHz instead of 1.2.
            warm_sb = wpool.tile([128, MS], BF16, name="warm_sb")
            nc.gpsimd.memset(warm_sb[:], 0.0)
            warm_ps = psq.tile([128, MS], F32, tag="q", name="warm_ps")
            for _ in range(16):
                nc.tensor.matmul(
                    warm_ps[:], warm_sb[:, 0:128], warm_sb[:], start=True, stop=True
                )

            w1_k = []
            xt0 = []
            for kc in range(KC):
                t = wpool.tile([128, E], BF16, tag=f"w1_{kc}", name=f"w1k{kc}")
                nc.sync.dma_start(t[:], w1_v[:, kc, :])
                w1_k.append(t)
                tx = xpool.tile([128, MS], BF16, tag=f"xt_{kc}", name=f"xt0_{kc}")
                nc.sync.dma_start(tx[:], xt_v[:, kc, 0:MS])
                xt0.append(tx)
            sel_t = wpool.tile([128, NC_, HEADS], BF16, name="sel_t")
            nc.sync.dma_start(sel_t[:], sel[:, :].rearrange("p (c g) -> p c g", g=HEADS))

            w2_k = []
            for ci in range(NC_):
                t = wpool.tile([128, E], BF16, tag=f"w2_{ci}", name=f"w2k{ci}")
                nc.sync.dma_start(t[:], w2_v[:, ci, :])
                w2_k.append(t)
            selt_t = wpool.tile([128, NC_, 128], BF16, name="selt_t")
            nc.sync.dma_start(selt_t[:], selt[:, :].rearrange("p (c q) -> p c q", q=128))

            # Software pipeline over stripes: while stripe ms runs its
            # q-projection (mm1) + exp + head-sum on the PE, stripe ms-1's
            # normalization (rb broadcast matmul + DVE mul) and output
            # projection (mm2) are interleaved so the PE never waits on the
            # softmax chain.
            prev_u = None       # u tiles of stripe ms-1
            prev_rcp = None     # reciprocal head-sums of stripe ms-1 (bf16)
            prev_ms = -1

            def emit_norm(pu, prcp):
                """rb broadcast matmuls (PE, contiguous block, K padded to 128
                so LDWEIGHTS overlaps like the main GEMM blocks) + DVE muls."""
                ats = []
                for ci in range(NC_):
                    rb_ps = psrb.tile([128, MS], F32, tag="rb", name="rb_ps")
                    nc.tensor.matmul(
                        rb_ps[:], selt_t[:, ci, :], prcp[:], start=True, stop=True
                    )
                    at_t = apool.tile([128, MS], BF16, tag="at", name="at_t")
                    nc.vector.tensor_mul(at_t[:], pu[ci][:], rb_ps[:])
                    ats.append(at_t)
                return ats

            def emit_tail(at_list, ms):
                """Emit mm2 + store for a finished stripe (at tiles ready)."""
                for j in range(NC_):
                    o_ps = pso.tile([128, MS], F32, tag="o", name="o_ps")
                    for ci in range(NC_):
                        nc.tensor.matmul(
                            o_ps[:],
                            w2_k[ci][:, j * 128:(j + 1) * 128],
                            at_list[ci][:],
                            start=(ci == 0),
                            stop=(ci == NC_ - 1),
                        )
                    o_t = opool.tile([128, MS], BF16, tag="ost", name="o_t")
                    nc.scalar.copy(o_t[:], o_ps[:])
                    nc.sync.dma_start(
                        outT[j * 128:(j + 1) * 128, ms * MS:(ms + 1) * MS], o_t[:]
                    )

            for ms in range(N_STRIPES):
                if ms == 0:
                    xt_k = xt0
                else:
                    xt_k = []
                    for kc in range(KC):
                        t = xpool.tile(
                            [128, MS], BF16, tag=f"xt_{kc}", name=f"xt{ms}_{kc}"
                        )
                        nc.sync.dma_start(
                            t[:], xt_v[:, kc, ms * MS:(ms + 1) * MS]
                        )
                        xt_k.append(t)

                # ---- mm1: q-projection, contiguous 64-MM block on PE ----
                u_tiles = []
                q_list = []
                for ci in range(NC_):
                    q_ps = psq.tile([128, MS], F32, tag="q", name="q_ps")
                    for kc in range(KC):
                        nc.tensor.matmul(
                            q_ps[:],
                            w1_k[kc][:, ci * 128:(ci + 1) * 128],
                            xt_k[kc][:],
                            start=(kc == 0),
                            stop=(kc == KC - 1),
                        )
                    u_t = upool.tile([128, MS], BF16, tag="u", name="u_t")
                    nc.scalar.activation(u_t[:], q_ps[:], AF.Exp, scale=0.25)
                    u_tiles.append(u_t)

                # ---- stripe ms-1 normalization (hides exp latency) ----
                at_tiles = emit_norm(prev_u, prev_rcp) if prev_rcp is not None else None

                # ---- head sums (contiguous 8-MM block) + reciprocal ----
                s_ps = pss.tile([HEADS, MS], F32, tag="s", name="s_ps")
                for ci in range(NC_):
                    nc.tensor.matmul(
                        s_ps[:],
                        sel_t[:, ci, :],
                        u_tiles[ci][:],
                        start=(ci == 0),
                        stop=(ci == NC_ - 1),
                    )
                rcp32 = spool.tile([HEADS, MS], F32, tag="rcp32", name="rcp32")
                nc.vector.reciprocal_approx_fast(rcp32[:], s_ps[:])
                # rcp padded to 128 partitions (rows 16+ zeroed on the idle
                # GpSimd engine) so the rb matmul runs with K=128
                rcp_t = spool.tile([128, MS], BF16, tag="rcp", name="rcp_t")
                nc.gpsimd.memset(rcp_t[:], 0.0)
                nc.scalar.copy(rcp_t[0:HEADS, :], rcp32[:])

                # ---- stripe ms-1 output projection ----
                if at_tiles is not None:
                    emit_tail(at_tiles, prev_ms)
                prev_u, prev_rcp, prev_ms = u_tiles, rcp_t, ms

            # epilogue: last stripe's normalization + output projection
            at_tiles = emit_norm(prev_u, prev_rcp)
            emit_tail(at_tiles, prev_ms)
    nc.compile()
    return nc


_NC_CACHE = None
LAST_RESULT = None


def _ensure_ntff_hook():
    """bass_utils' axon trace path needs antenv.axon_hooks, which this
    container's antenv lacks. Provide it + register the ctypes NTFF hook."""
    import types

    try:
        from antenv.axon_hooks import get_axon_ntff_profile_hook  # noqa: F401
        return True
    except ImportError:
        pass
    try:
        import antenv
        from trn_agent_boot.trn_boot import _ntff_profile_via_ctypes

        m = types.ModuleType("antenv.axon_hooks")
        state = {"hook": None}
        m.set_axon_ntff_profile_hook = lambda h: state.__setitem__("hook", h)
        m.get_axon_ntff_profile_hook = lambda: state["hook"]
        sys.modules["antenv.axon_hooks"] = m
        antenv.axon_hooks = m
        m.set_axon_ntff_profile_hook(
            _ntff_profile_via_ctypes("/opt/axon/libaxon_pjrt.so")
        )
        return True
    except Exception as e:  # pragma: no cover
        print(f"ntff hook injection failed: {e}")
        return False


def _selectors():
    # head index of global feature n is n // 64; chunk ci covers n in
    # [128ci, 128ci+128) -> heads 2ci (partitions 0..63) and 2ci+1 (64..127)
    sel = np.zeros((128, NC_, HEADS), np.float32)
    selt = np.zeros((128, NC_, 128), np.float32)  # K padded to 128, rows 16+ zero
    for ci in range(NC_):
        sel[:64, ci, 2 * ci] = 1.0
        sel[64:, ci, 2 * ci + 1] = 1.0
        selt[2 * ci, ci, :64] = 1.0
        selt[2 * ci + 1, ci, 64:] = 1.0
    return (
        np.ascontiguousarray(sel.reshape(128, NC_ * HEADS)).astype(_BF),
        np.ascontiguousarray(selt.reshape(128, NC_ * 128)).astype(_BF),
    )


def kernel(x, W1, W2, heads, trace=False):
    global _NC_CACHE, LAST_RESULT
    x = np.asarray(x, dtype=np.float32)
    W1 = np.asarray(W1, dtype=np.float32)
    W2 = np.asarray(W2, dtype=np.float32)

    X = x.reshape(M_TOTAL, E)
    Xbf = X.astype(_BF)
    XbfT = Xbf.T  # [E, M_TOTAL] view
    w1t = np.ascontiguousarray(W1[:E, :].T).astype(_BF)   # [k, n] = W1q[n, k]
    w2t = np.ascontiguousarray(W2.T).astype(_BF)          # [n, j] = W2[j, n]
    sel, selt = _selectors()

    in_maps = []
    for c in range(N_CORES):
        xt_c = np.ascontiguousarray(XbfT[:, c * M_CORE:(c + 1) * M_CORE])
        in_maps.append(
            {"xt": xt_c, "w1t": w1t, "w2t": w2t, "sel": sel, "selt": selt}
        )

    if _NC_CACHE is None:
        _NC_CACHE = build_nc()

    if trace:
        trace = _ensure_ntff_hook()

    res = run_bass_kernel_spmd(_NC_CACHE, in_maps, list(range(N_CORES)), trace=trace)
    LAST_RESULT = res

    OT = np.concatenate(
        [np.asarray(res.results[c]["outT"]).astype(np.float32) for c in range(N_CORES)],
        axis=1,
    )
    return np.ascontiguousarray(OT.T).reshape(B, S, E)



# revision 2
# speedup vs baseline: 1.5476x; 1.5476x over previous
"""Trainium2 Bass kernel for nn_Attention_9242769622327.

Math: the reference computes
    qkv = x @ W1.T ; q,k,v = split(qkv)
    score = softmax(k^T v / 4, axis=-1)            # rows sum to 1
    attn  = softmax(einsum('bhnk,bhkc->bhnk', q/4, score), axis=-1)
          = softmax(q/4 * sum_c score)             # sum_c score == 1
          = softmax(q/4)                           # k/v are mathematically dead
    out   = attn @ W2.T
so only the q-projection (first E rows of W1), a per-head (64-wide) softmax,
and the output projection are needed.

Distribution: pure data-parallel over the 32768 = B*S rows; each of the 8
cores handles 4096 rows with the full (transposed) weights. No collectives.

Both big GEMMs run in fp8 e4m3 with perf_mode=DoubleRow (2 fp8 MACs per PE
cell per cycle, K=256 per matmul).  fp8's ~2% rounding noise is kept out of
the output via two tricks:
  * mm1 noise enters pre-softmax and is damped 4x by the q/4 scale; weights
    are pre-scaled by 64 (W1q values ~N(0, 1/32^2) would land in e4m3's
    subnormal range) and the exp() activation scale absorbs the 1/64.
  * mm2 operates on d = 64*attn - 1 instead of attn: per head the 64 attn
    values sum to 1, so attn = 1/64 + small delta and quantizing the delta
    is 4x less noisy than quantizing attn.  out = W2 @ attn is reconstructed
    as out[j,m] = P[j,m]/4096 + S_j/64 with P = (64*W2)_fp8 @ d_fp8 and
    S_j = sum_n W2[j,n] computed exactly on the host (folded into the
    PSUM->SBUF copy as a per-partition bias).

On-chip layout is fully transposed (features on partitions, rows on the free
dim) so no on-chip transposes are needed anywhere:
    q64[n,m] = sum_k 64*W1qT[k,n] * xT[k,m]       (PE, fp8 DoubleRow)
    u        = exp(q64/256)                       (ACT, PSUM->SBUF fp16)
    s[g,m]   = sum_{n in head g} u[n,m]           (PE matmul w/ 0/1 selector)
    rcp      = 1/s                                (DVE reciprocal_approx_fast)
    rb64     = selt^T @ (64*rcp)                  (PE matmul w/ selector^T)
    at64     = u * rb64                           (DVE, fp16 = 64*attn)
    d8       = at64 - 1                           (DVE, fp8)
    P[j,m]   = sum_n (64*W2T)[n,j] * d8[n,m]      (PE, fp8 DoubleRow)
    outT     = P/4096 + S_j/64                    (ACT Identity w/ bias vec)

Stripes are software-pipelined: [8 rb(ms-1)][32 mm1(ms)][8 sel(ms)]
[32 mm2(ms-1)] as contiguous matmul blocks on the PE, with exp/reciprocal/
normalize/quantize hidden underneath on ACT/DVE.
"""

import sys

sys.path.insert(0, "/opt/trn_rl_repo")

import numpy as np
import ml_dtypes

import concourse.bass as bass
import concourse.bacc as bacc
import concourse.tile as tile
from concourse import mybir
from concourse.bass_utils import run_bass_kernel_spmd

F16 = mybir.dt.float16
F8 = mybir.dt.float8e4
F32 = mybir.dt.float32
AF = mybir.ActivationFunctionType
DR = mybir.MatmulPerfMode.DoubleRow

N_CORES = 8
B, S, E = 4, 8192, 1024
HEADS, HEAD_DIM = 16, 64
M_TOTAL = B * S                # 32768
M_CORE = M_TOTAL // N_CORES    # 4096 rows per core
MS = 512                       # m-stripe width (moving free dim / PSUM bank)
N_STRIPES = M_CORE // MS       # 8
KC2 = E // 256                 # 4 double-row contraction chunks
NC_ = E // 128                 # 8 feature chunks
WSCALE = 64.0                  # host pre-scale on W1q and W2

_NF16 = np.float16
_NF8 = ml_dtypes.float8_e4m3   # == TRN FP8_EXP4 (max normal 240, has inf)


def build_nc() -> bass.Bass:
    nc = bacc.Bacc("TRN2", debug=False)

    xt = nc.dram_tensor("xt", [E, M_CORE], F8, kind="ExternalInput")
    w1t = nc.dram_tensor("w1t", [E, E], F8, kind="ExternalInput")
    w2t = nc.dram_tensor("w2t", [E, E], F8, kind="ExternalInput")
    sel = nc.dram_tensor("sel", [128, NC_ * HEADS], F16, kind="ExternalInput")
    selt = nc.dram_tensor("selt", [128, NC_ * 128], F16, kind="ExternalInput")
    bias = nc.dram_tensor("bias", [128, NC_], F32, kind="ExternalInput")
    outT = nc.dram_tensor("outT", [E, M_CORE], F16, kind="ExternalOutput")

    # k = c*256 + i*128 + p  (DoubleRow pair plane i, partition p)
    xt_v = xt[:, :].rearrange("(c i p) m -> p c i m", p=128, i=2)
    w1_v = w1t[:, :].rearrange("(c i p) n -> p c i n", p=128, i=2)
    w2_v = w2t[:, :].rearrange("(c i p) j -> p c i j", p=128, i=2)

    with tile.TileContext(nc) as tc:
        with (
            tc.tile_pool(name="weights", bufs=1) as wpool,
            tc.tile_pool(name="xt", bufs=3) as xpool,
            tc.tile_pool(name="u", bufs=16) as upool,
            tc.tile_pool(name="at", bufs=16) as apool,
            tc.tile_pool(name="d8", bufs=8) as dpool,
            tc.tile_pool(name="small", bufs=3) as spool,
            tc.tile_pool(name="ostage", bufs=8) as opool,
            tc.tile_pool(name="ps_q", bufs=2, space="PSUM") as psq,
            tc.tile_pool(name="ps_s", bufs=2, space="PSUM") as pss,
            tc.tile_pool(name="ps_rb", bufs=2, space="PSUM") as psrb,
            tc.tile_pool(name="ps_o", bufs=2, space="PSUM") as pso,
        ):
            # Warm the PE's HAM clock gate with throwaway matmuls on memset
            # scratch while the first weight/x DMAs are in flight, so the
            # first real matmuls run at 2.4 GHz instead of 1.2.
            warm_sb = wpool.tile([128, MS], F16, name="warm_sb")
            nc.gpsimd.memset(warm_sb[:], 0.0)
            warm_ps = psq.tile([128, MS], F32, tag="q", name="warm_ps")
            for _ in range(16):
                nc.tensor.matmul(
                    warm_ps[:], warm_sb[:, 0:128], warm_sb[:], start=True, stop=True
                )

            # Per-chunk weight tiles so the first matmuls only wait on the
            # chunks they read, not the whole 2MB of weights.  Load order:
            # w1 + sel (needed by stripe 0's mm1/sel), stripe-0 x chunks,
            # then w2 + selt + bias (not needed until ~10us in).
            w1_k = []
            xt0 = []
            for c in range(KC2):
                t = wpool.tile([128, 2, E], F8, tag=f"w1_{c}", name=f"w1k{c}")
                nc.sync.dma_start(t[:], w1_v[:, c, :, :])
                w1_k.append(t)
                tx = xpool.tile([128, 2, MS], F8, tag=f"xt_{c}", name=f"xt0_{c}")
                nc.sync.dma_start(tx[:], xt_v[:, c, :, 0:MS])
                xt0.append(tx)
            sel_t = wpool.tile([128, NC_, HEADS], F16, name="sel_t")
            nc.sync.dma_start(sel_t[:], sel[:, :].rearrange("p (c g) -> p c g", g=HEADS))

            w2_k = []
            for c in range(KC2):
                t = wpool.tile([128, 2, E], F8, tag=f"w2_{c}", name=f"w2k{c}")
                nc.sync.dma_start(t[:], w2_v[:, c, :, :])
                w2_k.append(t)
            selt_t = wpool.tile([128, NC_, 128], F16, name="selt_t")
            nc.sync.dma_start(selt_t[:], selt[:, :].rearrange("p (c q) -> p c q", q=128))
            bias_t = wpool.tile([128, NC_], F32, name="bias_t")
            nc.sync.dma_start(bias_t[:], bias[:, :])

            # Software pipeline over stripes: stripe ms runs
            #   [rb(ms-1)][mm1(ms)][sel(ms)][mm2(ms-1)]
            # on the PE; the rb->at64->d8 chain (DVE) for stripe ms-1 hides
            # under mm1(ms), and exp(ms) (ACT) lands just after mm1(ms).
            prev_u = None       # u tiles of stripe ms-1
            prev_rcp = None     # 64/s head-sums of stripe ms-1 (fp16, padded)
            prev_ms = -1

            def emit_norm(pu, prcp):
                """rb broadcast matmuls (PE, contiguous block, K padded to 128
                so LDWEIGHTS overlaps) + DVE normalize/quantize to d8."""
                d_tiles = [
                    dpool.tile([128, 2, MS], F8, tag=f"d8_{c}", name=f"d8_{c}")
                    for c in range(KC2)
                ]
                for ci in range(NC_):
                    rb_ps = psrb.tile([128, MS], F32, tag="rb", name="rb_ps")
                    nc.tensor.matmul(
                        rb_ps[:], selt_t[:, ci, :], prcp[:], start=True, stop=True
                    )
                    at_t = apool.tile([128, MS], F16, tag="at", name="at_t")
                    nc.vector.tensor_mul(at_t[:], pu[ci][:], rb_ps[:])
                    nc.vector.tensor_scalar_sub(
                        d_tiles[ci // 2][:, ci % 2, :], at_t[:], 1.0
                    )
                return d_tiles

            def emit_tail(d_tiles, ms):
                """Emit mm2 + store for a finished stripe (d8 tiles ready)."""
                for j in range(NC_):
                    o_ps = pso.tile([128, MS], F32, tag="o", name="o_ps")
                    for c in range(KC2):
                        nc.tensor.matmul(
                            o_ps[:],
                            w2_k[c][:, :, j * 128:(j + 1) * 128],
                            d_tiles[c][:],
                            start=(c == 0),
                            stop=(c == KC2 - 1),
                            perf_mode=DR,
                        )
                    o_t = opool.tile([128, MS], F16, tag="ost", name="o_t")
                    nc.scalar.activation(
                        o_t[:], o_ps[:], AF.Identity,
                        bias=bias_t[:, j:j + 1], scale=1.0 / (WSCALE * WSCALE),
                    )
                    nc.sync.dma_start(
                        outT[j * 128:(j + 1) * 128, ms * MS:(ms + 1) * MS], o_t[:]
                    )

            for ms in range(N_STRIPES):
                if ms == 0:
                    xt_k = xt0
                else:
                    xt_k = []
                    for c in range(KC2):
                        t = xpool.tile(
                            [128, 2, MS], F8, tag=f"xt_{c}", name=f"xt{ms}_{c}"
                        )
                        nc.sync.dma_start(
                            t[:], xt_v[:, c, :, ms * MS:(ms + 1) * MS]
                        )
                        xt_k.append(t)

                # ---- stripe ms-1 normalization (rb block first: its d8
                # chain then hides under mm1 of this stripe) ----
                d_tiles = emit_norm(prev_u, prev_rcp) if prev_rcp is not None else None

                # ---- mm1: q-projection, contiguous 32-MM DoubleRow block ----
                u_tiles = []
                for ci in range(NC_):
                    q_ps = psq.tile([128, MS], F32, tag="q", name="q_ps")
                    for c in range(KC2):
                        nc.tensor.matmul(
                            q_ps[:],
                            w1_k[c][:, :, ci * 128:(ci + 1) * 128],
                            xt_k[c][:],
                            start=(c == 0),
                            stop=(c == KC2 - 1),
                            perf_mode=DR,
                        )
                    u_t = upool.tile([128, MS], F16, tag="u", name="u_t")
                    nc.scalar.activation(u_t[:], q_ps[:], AF.Exp, scale=0.25 / WSCALE)
                    u_tiles.append(u_t)

                # ---- head sums (contiguous 8-MM block) + reciprocal ----
                s_ps = pss.tile([HEADS, MS], F32, tag="s", name="s_ps")
                for ci in range(NC_):
                    nc.tensor.matmul(
                        s_ps[:],
                        sel_t[:, ci, :],
                        u_tiles[ci][:],
                        start=(ci == 0),
                        stop=(ci == NC_ - 1),
                    )
                rcp32 = spool.tile([HEADS, MS], F32, tag="rcp32", name="rcp32")
                nc.vector.reciprocal_approx_fast(rcp32[:], s_ps[:])
                # 64/s padded to 128 partitions (rows 16+ zeroed on the idle
                # GpSimd engine) so the rb matmul runs with K=128
                rcp_t = spool.tile([128, MS], F16, tag="rcp", name="rcp_t")
                nc.gpsimd.memset(rcp_t[:], 0.0)
                nc.scalar.activation(
                    rcp_t[0:HEADS, :], rcp32[:], AF.Copy, scale=WSCALE
                )

                # ---- stripe ms-1 output projection ----
                if d_tiles is not None:
                    emit_tail(d_tiles, prev_ms)
                prev_u, prev_rcp, prev_ms = u_tiles, rcp_t, ms

            # epilogue: last stripe's normalization + output projection
            d_tiles = emit_norm(prev_u, prev_rcp)
            emit_tail(d_tiles, prev_ms)
    nc.compile()
    return nc


_NC_CACHE = None
LAST_RESULT = None


def _ensure_ntff_hook():
    """bass_utils' axon trace path needs antenv.axon_hooks, which this
    container's antenv lacks. Provide it + register the ctypes NTFF hook."""
    import types

    try:
        from antenv.axon_hooks import get_axon_ntff_profile_hook  # noqa: F401
        return True
    except ImportError:
        pass
    try:
        import antenv
        from trn_agent_boot.trn_boot import _ntff_profile_via_ctypes

        m = types.ModuleType("antenv.axon_hooks")
        state = {"hook": None}
        m.set_axon_ntff_profile_hook = lambda h: state.__setitem__("hook", h)
        m.get_axon_ntff_profile_hook = lambda: state["hook"]
        sys.modules["antenv.axon_hooks"] = m
        antenv.axon_hooks = m
        m.set_axon_ntff_profile_hook(
            _ntff_profile_via_ctypes("/opt/axon/libaxon_pjrt.so")
        )
        return True
    except Exception as e:  # pragma: no cover
        print(f"ntff hook injection failed: {e}")
        return False


def _selectors():
    # head index of global feature n is n // 64; chunk ci covers n in
    # [128ci, 128ci+128) -> heads 2ci (partitions 0..63) and 2ci+1 (64..127)
    sel = np.zeros((128, NC_, HEADS), np.float32)
    selt = np.zeros((128, NC_, 128), np.float32)  # K padded to 128, rows 16+ zero
    for ci in range(NC_):
        sel[:64, ci, 2 * ci] = 1.0
        sel[64:, ci, 2 * ci + 1] = 1.0
        selt[2 * ci, ci, :64] = 1.0
        selt[2 * ci + 1, ci, 64:] = 1.0
    return (
        np.ascontiguousarray(sel.reshape(128, NC_ * HEADS)).astype(_NF16),
        np.ascontiguousarray(selt.reshape(128, NC_ * 128)).astype(_NF16),
    )


def kernel(x, W1, W2, heads, trace=False):
    global _NC_CACHE, LAST_RESULT
    x = np.asarray(x, dtype=np.float32)
    W1 = np.asarray(W1, dtype=np.float32)
    W2 = np.asarray(W2, dtype=np.float32)

    X = x.reshape(M_TOTAL, E)
    X8T = X.astype(_NF8).T  # [E, M_TOTAL] view
    w1t = np.ascontiguousarray(W1[:E, :].T * WSCALE).astype(_NF8)  # 64*W1q[n,k]^T
    w2t = np.ascontiguousarray(W2.T * WSCALE).astype(_NF8)         # 64*W2[j,n]^T
    sel, selt = _selectors()
    # bias[p, j] = S_{j*128+p} / 64 with S_j = sum_n W2[j, n] (exact fp32)
    bias = np.ascontiguousarray(
        (W2.sum(axis=1) / WSCALE).reshape(NC_, 128).T
    ).astype(np.float32)

    in_maps = []
    for c in range(N_CORES):
        xt_c = np.ascontiguousarray(X8T[:, c * M_CORE:(c + 1) * M_CORE])
        in_maps.append(
            {"xt": xt_c, "w1t": w1t, "w2t": w2t, "sel": sel, "selt": selt,
             "bias": bias}
        )

    if _NC_CACHE is None:
        _NC_CACHE = build_nc()

    if trace:
        trace = _ensure_ntff_hook()

    res = run_bass_kernel_spmd(_NC_CACHE, in_maps, list(range(N_CORES)), trace=trace)
    LAST_RESULT = res

    OT = np.concatenate(
        [np.asarray(res.results[c]["outT"]).astype(np.float32) for c in range(N_CORES)],
        axis=1,
    )
    return np.ascontiguousarray(OT.T).reshape(B, S, E)
